# revision 1
# baseline (speedup 1.0000x reference)
"""Trainium2 Bass kernel for Conv1D(SAME) + BN + ReLU -> LocallyConnected1D + BN + ReLU.

Sharding: sequence-parallel over output positions. Core i owns output
positions [64*i, 64*i + 64) (core 7 is zero-padded past position 505).
Each core reads only its slice of local_w (the dominant tensor), so
total HBM traffic stays at the single-read minimum. No collectives.

The kernel is HBM-stream-bound on local_w; everything is arranged so
that stream never stalls and nothing else sits on it:
  * local_w is host-transposed into the exact SBUF tile layout, so every
    transfer is one fully-contiguous HBM read (the previous on-device
    rearrange generated 512B-chunk descriptor storms: ~60x slower).
  * weights ship as bf16 (halves the stream; rel err ~3e-3 << 2e-2).
  * bulk pairs ride 4-pair (1.8 MB) transfers; the last 4 pairs ride
    2/1/1-pair transfers so the final position-block can finish sooner.
  * z is accumulated in SBUF (bf16) and leaves in one tail DMA instead
    of 16 small writes interleaved with the stream.

Host-side pre-processing folds both BatchNorms into the weights:
  y  = relu(conv(x) @ (conv_w * s1) + b1'),   s1 = g1*rsqrt(v1+eps)
  z  = relu(patches @ (local_w * s2) + b2'),  s2 = g2*rsqrt(v2+eps)
and lays x out transposed ([Cin, pos, batch]) so the conv contraction
dim is on SBUF partitions without any on-device transposes. Conv taps
are paired ([wc[2t]; wc[2t+1]] stationaries against a shift-duplicated
x) so the conv costs 4 instead of 7 matmuls per block.

local_w is pre-interleaved per position-pair so that the two chunks
needed at a given y-position q are adjacent in SBUF, giving N=256
matmuls. PSUM sub-slots are pair-swapped; the host unpermutes.
"""

import numpy as np

_B, _L, _CIN, _F, _K = 64, 512, 64, 128, 7
_OUT_LEN = _L - _K + 1  # 506
_NCORES = 8
_C = 64              # output positions per core (padded)
_NPAIR = _C // 2     # 32 position pairs
_NJB = 9             # conv j-blocks of 8 -> covers y positions [0, 72)
_LX = _NJB * 8 + 6   # 78 x positions per core (with halo + SAME pad)
_EPS = 1e-3
_WBUFS = 8           # in-flight 4-pair local_w tiles
_MODE = "bf16"       # "f32" | "f32r" | "bf16"


def _np_dt(mode):
    if mode == "bf16":
        import ml_dtypes
        return ml_dtypes.bfloat16
    return np.float32


def _build_program(bias_en: bool, mode: str | None = None, reps: int = 1):
    mode = mode or _MODE
    import concourse.mybir as mybir
    import concourse.tile as tile
    from concourse import bacc

    f32 = mybir.dt.float32
    # storage dtype for matmul operands: walrus requires FP32r consumers to
    # read locations *written* as FP32r, so declare end-to-end, no bitcast.
    dt_st = {"bf16": mybir.dt.bfloat16, "f32r": mybir.dt.float32r}.get(mode, f32)
    cast = lambda ap: ap

    nc = bacc.Bacc("TRN2", target_bir_lowering=False, debug=False)

    # xt: tap-shift duplicated — rows 0:63 = x[pos j], rows 64:127 = x[pos j+1]
    xt_d = nc.dram_tensor("xt", [2 * _CIN, _LX * _B], dt_st,
                          kind="ExternalInput")
    # wc: tap-paired — col block t holds [wc[2t]; wc[2t+1]] (block 3: [wc[6]; 0])
    wc_d = nc.dram_tensor("wc", [2 * _CIN, 4 * _F], dt_st, kind="ExternalInput")
    b1_d = nc.dram_tensor("b1", [_F, 1], f32, kind="ExternalInput")
    # pre-transposed on host so each tile is one fully-contiguous HBM read.
    # Bulk: 7 groups of 4 pairs; tail: 2+1+1 pairs in shrinking transfers so
    # the final position-block's weights land (and the kernel can end) sooner.
    wla_d = nc.dram_tensor("wla", [7, _F, 4 * 2 * _K * _F], dt_st,
                           kind="ExternalInput")
    wlb_d = nc.dram_tensor("wlb", [_F, 2 * 2 * _K * _F], dt_st,
                           kind="ExternalInput")
    wlc_d = nc.dram_tensor("wlc", [_F, 2 * _K * _F], dt_st,
                           kind="ExternalInput")
    # final pair ships as two 7-chunk halves (no matmul straddles chunk 7)
    # so the very last dependency is a half-size transfer
    wld_d = nc.dram_tensor("wld", [2, _F, _K * _F], dt_st,
                           kind="ExternalInput")
    if bias_en:
        b2_d = nc.dram_tensor("b2", [1, _C * _F], dt_st, kind="ExternalInput")
    zdt = dt_st if mode == "bf16" else f32
    z_d = nc.dram_tensor("z", [_B, _C * _F], zdt, kind="ExternalOutput")

    Relu = mybir.ActivationFunctionType.Relu

    with tile.TileContext(nc) as tc:
        with (
            tc.tile_pool(name="const", bufs=1) as cpool,
            tc.tile_pool(name="xt", bufs=1) as xpool,
            tc.tile_pool(name="yt", bufs=1) as ypool,
            tc.tile_pool(name="wt", bufs=_WBUFS) as wpool,
            tc.tile_pool(name="wt2", bufs=2) as wpool2,
            # bias_en adds the 32KB/partition b2 row tile; drop zb double-
            # buffering to stay inside SBUF in that (untriggered here) case
            tc.tile_pool(name="zst", bufs=1 if bias_en else 2) as zpool,
            tc.tile_pool(name="psc", bufs=2, space="PSUM") as pscpool,
            tc.tile_pool(name="psl", bufs=4, space="PSUM") as pslpool,
        ):
            # ---- constants / inputs to SBUF ----
            # xt rides nc.sync ahead of the wl stream; tiny constant loads go
            # on nc.scalar so their issue latency overlaps the sync stream.
            wc_t = cpool.tile([2 * _CIN, 4 * _F], dt_st)
            nc.scalar.dma_start(wc_t[:], wc_d[:])
            b1_t = cpool.tile([_F, 1], f32)
            nc.scalar.dma_start(b1_t[:], b1_d[:])
            if bias_en:
                b2_t = cpool.tile([1, _C * _F], dt_st)
                nc.scalar.dma_start(b2_t[:], b2_d[:])
                ones_t = cpool.tile([1, _B], dt_st)
                nc.gpsimd.memset(ones_t[:], 1.0)

            xt_t = xpool.tile([2 * _CIN, _LX * _B], dt_st)
            nxc = 2
            xch = (_LX * _B) // nxc
            for c in range(nxc):
                nc.sync.dma_start(
                    xt_t[:, c * xch:(c + 1) * xch],
                    xt_d[:, c * xch:(c + 1) * xch]
                )

            for r in range(reps):
                # ---- W stream (the big DMA) ----
                wgrps = []
                for gg in range(7):
                    wt = wpool.tile([_F, 4 * 2 * _K * _F], dt_st, tag="wt",
                                    name=f"wt{r}_{gg}")
                    nc.sync.dma_start(wt[:], wla_d[gg])
                    wgrps.append(wt)
                wtb = wpool2.tile([_F, 2 * 2 * _K * _F], dt_st, tag="wtb",
                                  name=f"wtb{r}")
                nc.sync.dma_start(wtb[:], wlb_d[:])
                wtc = wpool2.tile([_F, 2 * _K * _F], dt_st, tag="wtc",
                                  name=f"wtc{r}")
                nc.sync.dma_start(wtc[:], wlc_d[:])
                wtd = wpool2.tile([_F, 2 * _K * _F], dt_st, tag="wtd",
                                  name=f"wtd{r}")
                nc.sync.dma_start(wtd[:, :_K * _F], wld_d[0])
                nc.sync.dma_start(wtd[:, _K * _F:], wld_d[1])

                def wl_ap(g, c0, ncol):
                    if g < 28:
                        t_, base = wgrps[g // 4], (g % 4) * 2 * _K
                    elif g < 30:
                        t_, base = wtb, (g - 28) * 2 * _K
                    else:
                        t_, base = (wtc if g == 30 else wtd), 0
                    return t_[:, (base + c0) * _F:(base + c0 + ncol) * _F]

                # ---- conv + BN1 + ReLU -> yT [F, (j, b)] ----
                # taps paired two-per-matmul: stationary [wc[2t]; wc[2t+1]]
                # (128 rows), moving xt col j = [x[j]; x[j+1]] -> 4 mms per jb.
                yt_t = ypool.tile([_F, _NJB * 8 * _B], dt_st, tag="yt",
                                  name=f"yt{r}")
                for jb in range(_NJB):
                    ps = pscpool.tile([_F, 8 * _B], f32, tag="psc",
                                      name=f"psc{r}_{jb}")
                    for t in range(4):
                        nc.tensor.matmul(
                            ps[:],
                            cast(wc_t[:, t * _F:(t + 1) * _F]),
                            cast(xt_t[:, (8 * jb + 2 * t) * _B:
                                      (8 * jb + 2 * t + 8) * _B]),
                            start=(t == 0),
                            stop=(t == 3),
                        )
                    nc.scalar.activation(
                        yt_t[:, jb * 8 * _B:(jb + 1) * 8 * _B], ps[:], Relu,
                        bias=b1_t[:]
                    )

                # ---- locally-connected layer ----
                # bank-blocked: positions [4t, 4t+4) share one PSUM bank and
                # one accumulation group (HW start=True zeroes the whole 2KB
                # bank). wl cols: c = 2k + (p%2); at stationary q the active
                # chunks of a pair are adjacent -> one N=256 matmul. PSUM
                # sub-slot of local position j is j^1; host unpermutes.
                zb_t = zpool.tile([_B, _C * _F], zdt, tag="zb", name=f"zb{r}")
                for t in range(_C // 4):
                    ps = pslpool.tile([_B, 4 * _F], f32, tag="psl",
                                      name=f"psl{r}_{t}")
                    # singles first: the start=True MM marks the whole 2KB bank
                    # pending; the other three singles land in fully-pending
                    # slots; every later paired MM then touches only
                    # already-written bytes (uniform accumulate).
                    mms = [  # (q, g, col_lo, ncols, out_lo)
                        (4 * t, 2 * t, 0, 1, 1),
                        (4 * t + _K, 2 * t, 2 * _K - 1, 1, 0),
                        (4 * t + 2, 2 * t + 1, 0, 1, 3),
                        (4 * t + 2 + _K, 2 * t + 1, 2 * _K - 1, 1, 2),
                    ]
                    # paired MMs grouped by pair (g) so everything needing only
                    # the earlier-arriving pair runs before the later pair's
                    # DMA lands — shrinks the wait on the final position-block.
                    for g in (2 * t, 2 * t + 1):
                        for q in range(4 * t, 4 * t + 10):
                            ke, ko = q - 2 * g, q - 2 * g - 1
                            if 0 <= ko and ke < _K:      # both chunks active
                                mms.append((q, g, 2 * ke - 1, 2, 2 * g - 4 * t))
                    for i, (q, g, c0, ncol, u0) in enumerate(mms):
                        nc.tensor.matmul(
                            ps[:, u0 * _F:(u0 + ncol) * _F],
                            cast(yt_t[:, q * _B:(q + 1) * _B]),
                            cast(wl_ap(g, c0, ncol)),
                            start=(i == 0),
                            stop=(i == len(mms) - 1) and not bias_en,
                        )
                    base = 4 * t
                    if bias_en:
                        nc.tensor.matmul(
                            ps[:],
                            cast(ones_t[:, :_B]),
                            cast(b2_t[:, base * _F:(base + 4) * _F]),
                            start=False,
                            stop=True,
                            skip_group_check=True,
                        )
                    nc.scalar.activation(
                        zb_t[:, base * _F:(base + 4) * _F], ps[:], Relu)
                    # z leaves in three chunks (positions 0-31, 32-55, 56-63):
                    # the first two hide mid-kernel; only 0.13 MB rides the
                    # tail behind the last matmul's activation.
                    half, c2 = (_C // 2) * _F, (_C - 8) * _F
                    if t == _C // 8 - 1:
                        nc.scalar.dma_start(z_d[:, :half], zb_t[:, :half])
                    elif t == _C // 4 - 3:
                        nc.scalar.dma_start(z_d[:, half:c2], zb_t[:, half:c2])
                c2 = (_C - 8) * _F
                nc.scalar.dma_start(z_d[:, c2:], zb_t[:, c2:])
    nc.compile()
    return nc


def _host_prepare(x, conv_w, conv_b, bn1_gamma, bn1_beta, bn1_mean, bn1_var,
                  local_w, local_b, bn2_gamma, bn2_beta, bn2_mean, bn2_var,
                  mode: str | None = None):
    mode = mode or _MODE
    f = np.float32
    dt = _np_dt(mode)
    x = np.asarray(x, f)
    s1 = (np.asarray(bn1_gamma, f) / np.sqrt(np.asarray(bn1_var, f) + f(_EPS))).astype(f)
    wc = (np.asarray(conv_w, f) * s1[None, None, :]).astype(f)
    b1 = (s1 * (np.asarray(conv_b, f) - np.asarray(bn1_mean, f))
          + np.asarray(bn1_beta, f)).astype(f).reshape(_F, 1)
    s2 = (np.asarray(bn2_gamma, f) / np.sqrt(np.asarray(bn2_var, f) + f(_EPS))).astype(f)
    wl = (np.asarray(local_w, f) * s2[None, None, :]).astype(f)
    b2 = (s2[None, :] * (np.asarray(local_b, f) - np.asarray(bn2_mean, f)[None, :])
          + np.asarray(bn2_beta, f)[None, :]).astype(f)

    bias_en = bool(np.any(b2))

    npad = _NCORES * _C  # 512
    # pair-interleaved + F-transposed local_w: per pair [f, (c=2k+(p%2), n)]
    # so any run of pairs is one contiguous HBM read into its SBUF layout.
    wl_pad = np.zeros((npad, _K, _F, _F), f)
    wl_pad[:_OUT_LEN] = wl.reshape(_OUT_LEN, _K, _F, _F)
    wl_pT = np.ascontiguousarray(
        wl_pad.reshape(npad // 2, 2, _K, _F, _F)
        .transpose(0, 3, 2, 1, 4)             # [pair, f, k, p, n]
    ).reshape(npad // 2, _F, 2 * _K * _F).astype(dt)

    perm = np.arange(_C) ^ 1  # pair-swap (self-inverse)
    b2_pad = np.zeros((npad, _F), f)
    b2_pad[:_OUT_LEN] = b2

    # x padded for SAME conv + per-core halo: xpad[:, j] = x[:, j-3]
    xpad = np.zeros((_B, _L + 3 + 16, _CIN), f)
    xpad[:, 3:3 + _L] = x
    xpad = xpad.astype(dt)

    # tap-paired conv weights: [2*CIN, 4F]; block t = [wc[2t]; wc[2t+1]]
    wc128 = np.zeros((2 * _CIN, 4, _F), f)
    for t in range(3):
        wc128[:_CIN, t] = wc[2 * t]
        wc128[_CIN:, t] = wc[2 * t + 1]
    wc128[:_CIN, 3] = wc[6]
    wc128 = np.ascontiguousarray(wc128.reshape(2 * _CIN, 4 * _F)).astype(dt)

    in_maps = []
    for i in range(_NCORES):
        p0 = _C * i
        xs = xpad[:, p0:p0 + _LX + 1, :]                  # [B, LX+1, CIN]
        xtT = xs.transpose(2, 1, 0)                       # [CIN, LX+1, B]
        xt = np.ascontiguousarray(
            np.concatenate([xtT[:, :_LX], xtT[:, 1:]], axis=0)
        ).reshape(2 * _CIN, _LX * _B)
        pr = wl_pT[p0 // 2:p0 // 2 + _NPAIR]              # [32, F, 2K*F]
        wla = np.ascontiguousarray(
            pr[:28].reshape(7, 4, _F, 2 * _K * _F).transpose(0, 2, 1, 3)
        ).reshape(7, _F, 4 * 2 * _K * _F)
        wlb = np.ascontiguousarray(
            pr[28:30].transpose(1, 0, 2)).reshape(_F, 2 * 2 * _K * _F)
        wld = np.ascontiguousarray(
            pr[31].reshape(_F, 2, _K * _F).transpose(1, 0, 2))
        m = {"xt": xt, "wc": wc128, "b1": b1,
             "wla": wla, "wlb": wlb, "wlc": pr[30], "wld": wld}
        if bias_en:
            m["b2"] = np.ascontiguousarray(
                b2_pad[p0:p0 + _C][perm].reshape(1, _C * _F)).astype(dt)
        in_maps.append(m)
    return in_maps, bias_en


def _assemble(results):
    f = np.float32
    perm = np.arange(_C) ^ 1
    z = np.empty((_B, _OUT_LEN, _F), f)
    for i in range(_NCORES):
        p0 = _C * i
        zi = np.asarray(results[i]["z"], f).reshape(_B, _C, _F)[:, perm]
        n = min(_C, _OUT_LEN - p0)
        z[:, p0:p0 + n] = zi[:, :n]
    return z


def kernel(**inputs) -> np.ndarray:
    from concourse.bass_utils import run_bass_kernel_spmd

    in_maps, bias_en = _host_prepare(**inputs)
    nc = _build_program(bias_en)
    res = run_bass_kernel_spmd(nc, in_maps, list(range(_NCORES)))
    return _assemble(res.results)



# revision 3
# speedup vs baseline: 58.3978x; 58.3978x over previous
"""Trainium2 Bass kernel for Conv1D(SAME) + BN + ReLU -> LocallyConnected1D + BN + ReLU.

Sharding: sequence-parallel over output positions. Core i owns output
positions [64*i, 64*i + 64) (core 7 is zero-padded past position 505).
Each core reads only its slice of local_w (the dominant tensor), so
total HBM traffic stays at the single-read minimum. No collectives.

The kernel is HBM-stream-bound on local_w; everything is arranged so
that stream never stalls and nothing else sits on it:
  * local_w is host-transposed into the exact SBUF tile layout, so every
    transfer is one fully-contiguous HBM read (the previous on-device
    rearrange generated 512B-chunk descriptor storms: ~60x slower).
  * weights ship as bf16 (halves the stream; rel err ~3e-3 << 2e-2).
  * bulk pairs ride 4-pair (1.8 MB) transfers; the last 4 pairs ride
    2/1/1-pair transfers so the final position-block can finish sooner.
  * z is accumulated in SBUF (bf16) and leaves in one tail DMA instead
    of 16 small writes interleaved with the stream.

Host-side pre-processing folds both BatchNorms into the weights:
  y  = relu(conv(x) @ (conv_w * s1) + b1'),   s1 = g1*rsqrt(v1+eps)
  z  = relu(patches @ (local_w * s2) + b2'),  s2 = g2*rsqrt(v2+eps)
and lays x out transposed ([Cin, pos, batch]) so the conv contraction
dim is on SBUF partitions without any on-device transposes. Conv taps
are paired ([wc[2t]; wc[2t+1]] stationaries against a shift-duplicated
x) so the conv costs 4 instead of 7 matmuls per block.

local_w is pre-interleaved per position-pair so that the two chunks
needed at a given y-position q are adjacent in SBUF, giving N=256
matmuls. PSUM sub-slots are pair-swapped; the host unpermutes.
"""

import numpy as np

_B, _L, _CIN, _F, _K = 64, 512, 64, 128, 7
_OUT_LEN = _L - _K + 1  # 506
_NCORES = 8
_C = 64              # output positions per core (padded)
_NPAIR = _C // 2     # 32 position pairs
_NJB = 9             # conv j-blocks of 8 -> covers y positions [0, 72)
_LX = _NJB * 8 + 6   # 78 x positions per core (with halo + SAME pad)
_EPS = 1e-3
_WBUFS = 8           # in-flight 4-pair local_w tiles
_MODE = "bf16"       # "f32" | "f32r" | "bf16"


def _np_dt(mode):
    if mode == "bf16":
        import ml_dtypes
        return ml_dtypes.bfloat16
    return np.float32


def _build_program(bias_en: bool, mode: str | None = None, reps: int = 1):
    mode = mode or _MODE
    import concourse.mybir as mybir
    import concourse.tile as tile
    from concourse import bacc

    f32 = mybir.dt.float32
    # storage dtype for matmul operands: walrus requires FP32r consumers to
    # read locations *written* as FP32r, so declare end-to-end, no bitcast.
    dt_st = {"bf16": mybir.dt.bfloat16, "f32r": mybir.dt.float32r}.get(mode, f32)
    cast = lambda ap: ap

    nc = bacc.Bacc("TRN2", target_bir_lowering=False, debug=False)

    # xt: tap-shift duplicated — rows 0:63 = x[pos j], rows 64:127 = x[pos j+1]
    xt_d = nc.dram_tensor("xt", [2 * _CIN, _LX * _B], dt_st,
                          kind="ExternalInput")
    # wc: tap-paired — col block t holds [wc[2t]; wc[2t+1]] (block 3: [wc[6]; 0])
    wc_d = nc.dram_tensor("wc", [2 * _CIN, 4 * _F], dt_st, kind="ExternalInput")
    b1_d = nc.dram_tensor("b1", [_F, 1], f32, kind="ExternalInput")
    # pre-transposed on host so each tile is one fully-contiguous HBM read.
    # Bulk: 7 groups of 4 pairs; tail: 2+1+1 pairs in shrinking transfers so
    # the final position-block's weights land (and the kernel can end) sooner.
    wla_d = nc.dram_tensor("wla", [7, _F, 4 * 2 * _K * _F], dt_st,
                           kind="ExternalInput")
    wlb_d = nc.dram_tensor("wlb", [_F, 2 * 2 * _K * _F], dt_st,
                           kind="ExternalInput")
    wlc_d = nc.dram_tensor("wlc", [_F, 2 * _K * _F], dt_st,
                           kind="ExternalInput")
    # final pair ships as two 7-chunk halves (no matmul straddles chunk 7)
    # so the very last dependency is a half-size transfer
    wld_d = nc.dram_tensor("wld", [2, _F, _K * _F], dt_st,
                           kind="ExternalInput")
    if bias_en:
        b2_d = nc.dram_tensor("b2", [1, _C * _F], dt_st, kind="ExternalInput")
    zdt = dt_st if mode == "bf16" else f32
    z_d = nc.dram_tensor("z", [_B, _C * _F], zdt, kind="ExternalOutput")

    Relu = mybir.ActivationFunctionType.Relu

    with tile.TileContext(nc) as tc:
        with (
            tc.tile_pool(name="const", bufs=2 if reps > 1 else 1) as cpool,
            tc.tile_pool(name="xt", bufs=2 if reps > 1 else 1) as xpool,
            tc.tile_pool(name="yt", bufs=1) as ypool,
            tc.tile_pool(name="wt", bufs=_WBUFS) as wpool,
            tc.tile_pool(name="wt2", bufs=2) as wpool2,
            # bias_en adds the 32KB/partition b2 row tile; drop zb double-
            # buffering to stay inside SBUF in that (untriggered here) case
            tc.tile_pool(name="zst", bufs=1 if bias_en else 2) as zpool,
            tc.tile_pool(name="psc", bufs=2, space="PSUM") as pscpool,
            tc.tile_pool(name="psl", bufs=4, space="PSUM") as pslpool,
        ):
            for r in range(reps):
                # ---- constants / inputs to SBUF (per rep: one rep == one
                # full kernel execution, every input re-read from HBM) ----
                # xt rides nc.sync ahead of the wl stream; tiny constant loads
                # go on nc.scalar so their issue latency overlaps the stream.
                wc_t = cpool.tile([2 * _CIN, 4 * _F], dt_st, tag="wc",
                                  name=f"wc{r}")
                nc.scalar.dma_start(wc_t[:], wc_d[:])
                b1_t = cpool.tile([_F, 1], f32, tag="b1", name=f"b1_{r}")
                nc.scalar.dma_start(b1_t[:], b1_d[:])
                if bias_en:
                    b2_t = cpool.tile([1, _C * _F], dt_st, tag="b2",
                                      name=f"b2_{r}")
                    nc.scalar.dma_start(b2_t[:], b2_d[:])
                    ones_t = cpool.tile([1, _B], dt_st, tag="ones",
                                        name=f"ones{r}")
                    nc.gpsimd.memset(ones_t[:], 1.0)

                xt_t = xpool.tile([2 * _CIN, _LX * _B], dt_st, tag="xt",
                                  name=f"xt{r}")
                nxc = 2
                xch = (_LX * _B) // nxc
                for c in range(nxc):
                    nc.sync.dma_start(
                        xt_t[:, c * xch:(c + 1) * xch],
                        xt_d[:, c * xch:(c + 1) * xch]
                    )

                # ---- W stream (the big DMA) ----
                wgrps = []
                for gg in range(7):
                    wt = wpool.tile([_F, 4 * 2 * _K * _F], dt_st, tag="wt",
                                    name=f"wt{r}_{gg}")
                    nc.sync.dma_start(wt[:], wla_d[gg])
                    wgrps.append(wt)
                wtb = wpool2.tile([_F, 2 * 2 * _K * _F], dt_st, tag="wtb",
                                  name=f"wtb{r}")
                nc.sync.dma_start(wtb[:], wlb_d[:])
                wtc = wpool2.tile([_F, 2 * _K * _F], dt_st, tag="wtc",
                                  name=f"wtc{r}")
                nc.sync.dma_start(wtc[:], wlc_d[:])
                wtd = wpool2.tile([_F, 2 * _K * _F], dt_st, tag="wtd",
                                  name=f"wtd{r}")
                nc.sync.dma_start(wtd[:, :_K * _F], wld_d[0])
                nc.sync.dma_start(wtd[:, _K * _F:], wld_d[1])

                def wl_ap(g, c0, ncol):
                    if g < 28:
                        t_, base = wgrps[g // 4], (g % 4) * 2 * _K
                    elif g < 30:
                        t_, base = wtb, (g - 28) * 2 * _K
                    else:
                        t_, base = (wtc if g == 30 else wtd), 0
                    return t_[:, (base + c0) * _F:(base + c0 + ncol) * _F]

                # ---- conv + BN1 + ReLU -> yT [F, (j, b)] ----
                # taps paired two-per-matmul: stationary [wc[2t]; wc[2t+1]]
                # (128 rows), moving xt col j = [x[j]; x[j+1]] -> 4 mms per jb.
                yt_t = ypool.tile([_F, _NJB * 8 * _B], dt_st, tag="yt",
                                  name=f"yt{r}")
                for jb in range(_NJB):
                    ps = pscpool.tile([_F, 8 * _B], f32, tag="psc",
                                      name=f"psc{r}_{jb}")
                    for t in range(4):
                        nc.tensor.matmul(
                            ps[:],
                            cast(wc_t[:, t * _F:(t + 1) * _F]),
                            cast(xt_t[:, (8 * jb + 2 * t) * _B:
                                      (8 * jb + 2 * t + 8) * _B]),
                            start=(t == 0),
                            stop=(t == 3),
                        )
                    nc.scalar.activation(
                        yt_t[:, jb * 8 * _B:(jb + 1) * 8 * _B], ps[:], Relu,
                        bias=b1_t[:]
                    )

                # ---- locally-connected layer ----
                # bank-blocked: positions [4t, 4t+4) share one PSUM bank and
                # one accumulation group (HW start=True zeroes the whole 2KB
                # bank). wl cols: c = 2k + (p%2); at stationary q the active
                # chunks of a pair are adjacent -> one N=256 matmul. PSUM
                # sub-slot of local position j is j^1; host unpermutes.
                zb_t = zpool.tile([_B, _C * _F], zdt, tag="zb", name=f"zb{r}")
                for t in range(_C // 4):
                    ps = pslpool.tile([_B, 4 * _F], f32, tag="psl",
                                      name=f"psl{r}_{t}")
                    # singles first: the start=True MM marks the whole 2KB bank
                    # pending; the other three singles land in fully-pending
                    # slots; every later paired MM then touches only
                    # already-written bytes (uniform accumulate).
                    mms = [  # (q, g, col_lo, ncols, out_lo)
                        (4 * t, 2 * t, 0, 1, 1),
                        (4 * t + _K, 2 * t, 2 * _K - 1, 1, 0),
                        (4 * t + 2, 2 * t + 1, 0, 1, 3),
                        (4 * t + 2 + _K, 2 * t + 1, 2 * _K - 1, 1, 2),
                    ]
                    # paired MMs grouped by pair (g) so everything needing only
                    # the earlier-arriving pair runs before the later pair's
                    # DMA lands — shrinks the wait on the final position-block.
                    for g in (2 * t, 2 * t + 1):
                        for q in range(4 * t, 4 * t + 10):
                            ke, ko = q - 2 * g, q - 2 * g - 1
                            if 0 <= ko and ke < _K:      # both chunks active
                                mms.append((q, g, 2 * ke - 1, 2, 2 * g - 4 * t))
                    for i, (q, g, c0, ncol, u0) in enumerate(mms):
                        nc.tensor.matmul(
                            ps[:, u0 * _F:(u0 + ncol) * _F],
                            cast(yt_t[:, q * _B:(q + 1) * _B]),
                            cast(wl_ap(g, c0, ncol)),
                            start=(i == 0),
                            stop=(i == len(mms) - 1) and not bias_en,
                        )
                    base = 4 * t
                    if bias_en:
                        nc.tensor.matmul(
                            ps[:],
                            cast(ones_t[:, :_B]),
                            cast(b2_t[:, base * _F:(base + 4) * _F]),
                            start=False,
                            stop=True,
                            skip_group_check=True,
                        )
                    nc.scalar.activation(
                        zb_t[:, base * _F:(base + 4) * _F], ps[:], Relu)
                    # z leaves in three chunks (positions 0-31, 32-55, 56-63):
                    # the first two hide mid-kernel; only 0.13 MB rides the
                    # tail behind the last matmul's activation.
                    half, c2 = (_C // 2) * _F, (_C - 8) * _F
                    if t == _C // 8 - 1:
                        nc.scalar.dma_start(z_d[:, :half], zb_t[:, :half])
                    elif t == _C // 4 - 3:
                        nc.scalar.dma_start(z_d[:, half:c2], zb_t[:, half:c2])
                c2 = (_C - 8) * _F
                nc.scalar.dma_start(z_d[:, c2:], zb_t[:, c2:])
    nc.compile()
    return nc


def _host_prepare(x, conv_w, conv_b, bn1_gamma, bn1_beta, bn1_mean, bn1_var,
                  local_w, local_b, bn2_gamma, bn2_beta, bn2_mean, bn2_var,
                  mode: str | None = None):
    mode = mode or _MODE
    f = np.float32
    dt = _np_dt(mode)
    x = np.asarray(x, f)
    s1 = (np.asarray(bn1_gamma, f) / np.sqrt(np.asarray(bn1_var, f) + f(_EPS))).astype(f)
    wc = (np.asarray(conv_w, f) * s1[None, None, :]).astype(f)
    b1 = (s1 * (np.asarray(conv_b, f) - np.asarray(bn1_mean, f))
          + np.asarray(bn1_beta, f)).astype(f).reshape(_F, 1)
    s2 = (np.asarray(bn2_gamma, f) / np.sqrt(np.asarray(bn2_var, f) + f(_EPS))).astype(f)
    wl = (np.asarray(local_w, f) * s2[None, None, :]).astype(f)
    b2 = (s2[None, :] * (np.asarray(local_b, f) - np.asarray(bn2_mean, f)[None, :])
          + np.asarray(bn2_beta, f)[None, :]).astype(f)

    bias_en = bool(np.any(b2))

    npad = _NCORES * _C  # 512
    # pair-interleaved + F-transposed local_w: per pair [f, (c=2k+(p%2), n)]
    # so any run of pairs is one contiguous HBM read into its SBUF layout.
    wl_pad = np.zeros((npad, _K, _F, _F), f)
    wl_pad[:_OUT_LEN] = wl.reshape(_OUT_LEN, _K, _F, _F)
    wl_pT = np.ascontiguousarray(
        wl_pad.reshape(npad // 2, 2, _K, _F, _F)
        .transpose(0, 3, 2, 1, 4)             # [pair, f, k, p, n]
    ).reshape(npad // 2, _F, 2 * _K * _F).astype(dt)

    perm = np.arange(_C) ^ 1  # pair-swap (self-inverse)
    b2_pad = np.zeros((npad, _F), f)
    b2_pad[:_OUT_LEN] = b2

    # x padded for SAME conv + per-core halo: xpad[:, j] = x[:, j-3]
    xpad = np.zeros((_B, _L + 3 + 16, _CIN), f)
    xpad[:, 3:3 + _L] = x
    xpad = xpad.astype(dt)

    # tap-paired conv weights: [2*CIN, 4F]; block t = [wc[2t]; wc[2t+1]]
    wc128 = np.zeros((2 * _CIN, 4, _F), f)
    for t in range(3):
        wc128[:_CIN, t] = wc[2 * t]
        wc128[_CIN:, t] = wc[2 * t + 1]
    wc128[:_CIN, 3] = wc[6]
    wc128 = np.ascontiguousarray(wc128.reshape(2 * _CIN, 4 * _F)).astype(dt)

    in_maps = []
    for i in range(_NCORES):
        p0 = _C * i
        xs = xpad[:, p0:p0 + _LX + 1, :]                  # [B, LX+1, CIN]
        xtT = xs.transpose(2, 1, 0)                       # [CIN, LX+1, B]
        xt = np.ascontiguousarray(
            np.concatenate([xtT[:, :_LX], xtT[:, 1:]], axis=0)
        ).reshape(2 * _CIN, _LX * _B)
        pr = wl_pT[p0 // 2:p0 // 2 + _NPAIR]              # [32, F, 2K*F]
        wla = np.ascontiguousarray(
            pr[:28].reshape(7, 4, _F, 2 * _K * _F).transpose(0, 2, 1, 3)
        ).reshape(7, _F, 4 * 2 * _K * _F)
        wlb = np.ascontiguousarray(
            pr[28:30].transpose(1, 0, 2)).reshape(_F, 2 * 2 * _K * _F)
        wld = np.ascontiguousarray(
            pr[31].reshape(_F, 2, _K * _F).transpose(1, 0, 2))
        m = {"xt": xt, "wc": wc128, "b1": b1,
             "wla": wla, "wlb": wlb, "wlc": pr[30], "wld": wld}
        if bias_en:
            m["b2"] = np.ascontiguousarray(
                b2_pad[p0:p0 + _C][perm].reshape(1, _C * _F)).astype(dt)
        in_maps.append(m)
    return in_maps, bias_en


def _assemble(results):
    f = np.float32
    perm = np.arange(_C) ^ 1
    z = np.empty((_B, _OUT_LEN, _F), f)
    for i in range(_NCORES):
        p0 = _C * i
        zi = np.asarray(results[i]["z"], f).reshape(_B, _C, _F)[:, perm]
        n = min(_C, _OUT_LEN - p0)
        z[:, p0:p0 + n] = zi[:, :n]
    return z


def kernel(**inputs) -> np.ndarray:
    from concourse.bass_utils import run_bass_kernel_spmd

    in_maps, bias_en = _host_prepare(**inputs)
    nc = _build_program(bias_en)
    res = run_bass_kernel_spmd(nc, in_maps, list(range(_NCORES)))
    return _assemble(res.results)



# revision 7
# speedup vs baseline: 62.5837x; 1.0717x over previous
"""Trainium2 Bass kernel for Conv1D(SAME) + BN + ReLU -> LocallyConnected1D + BN + ReLU.

Sharding: sequence-parallel over output positions. Core i owns output
positions [64*i, 64*i + 64) (core 7 is zero-padded past position 505).
Each core reads only its slice of local_w (the dominant tensor), so
total HBM traffic stays at the single-read minimum. No collectives.

The kernel is HBM-stream-bound on local_w; everything is arranged so
that stream never stalls and nothing else sits on it:
  * local_w is host-transposed into the exact SBUF tile layout, so every
    transfer is one fully-contiguous HBM read (the previous on-device
    rearrange generated 512B-chunk descriptor storms: ~60x slower).
  * weights ship as bf16 (halves the stream; rel err ~3e-3 << 2e-2).
  * bulk pairs ride 4-pair (1.8 MB) transfers; the last 4 pairs ride
    2/1/1-pair transfers so the final position-block can finish sooner.
  * z is accumulated in SBUF (bf16) and leaves in one tail DMA instead
    of 16 small writes interleaved with the stream.

Host-side pre-processing folds both BatchNorms into the weights:
  y  = relu(conv(x) @ (conv_w * s1) + b1'),   s1 = g1*rsqrt(v1+eps)
  z  = relu(patches @ (local_w * s2) + b2'),  s2 = g2*rsqrt(v2+eps)
and lays x out transposed ([Cin, pos, batch]) so the conv contraction
dim is on SBUF partitions without any on-device transposes. Conv taps
are paired ([wc[2t]; wc[2t+1]] stationaries against a shift-duplicated
x) so the conv costs 4 instead of 7 matmuls per block.

local_w is pre-interleaved per position-pair so that the two chunks
needed at a given y-position q are adjacent in SBUF, giving N=256
matmuls. PSUM sub-slots are pair-swapped; the host unpermutes.
"""

import numpy as np

_B, _L, _CIN, _F, _K = 64, 512, 64, 128, 7
_OUT_LEN = _L - _K + 1  # 506
_NCORES = 8
_C = 64              # output positions per core (padded)
_NPAIR = _C // 2     # 32 position pairs
_NJB = 9             # conv j-blocks of 8 -> covers y positions [0, 72)
_LX = _NJB * 8 + 6   # 78 x positions per core (with halo + SAME pad)
_EPS = 1e-3
_WBUFS = 8           # in-flight 4-pair local_w tiles
_MODE = "bf16"       # "f32" | "f32r" | "bf16"


def _np_dt(mode):
    if mode == "bf16":
        import ml_dtypes
        return ml_dtypes.bfloat16
    return np.float32


def _build_program(bias_en: bool, mode: str | None = None, reps: int = 1):
    mode = mode or _MODE
    import concourse.mybir as mybir
    import concourse.tile as tile
    from concourse import bacc

    f32 = mybir.dt.float32
    # storage dtype for matmul operands: walrus requires FP32r consumers to
    # read locations *written* as FP32r, so declare end-to-end, no bitcast.
    dt_st = {"bf16": mybir.dt.bfloat16, "f32r": mybir.dt.float32r}.get(mode, f32)
    cast = lambda ap: ap

    nc = bacc.Bacc("TRN2", target_bir_lowering=False, debug=False)

    # xt: single copy [CIN, (LX+1) pos, B]; the tap-shift duplicate (rows
    # 64:127 = x[pos j+1]) is built on-device by an SBUF->SBUF DMA so HBM
    # only ships x once.
    xt_d = nc.dram_tensor("xt", [_CIN, (_LX + 1) * _B], dt_st,
                          kind="ExternalInput")
    # wc: tap-paired — col block t holds [wc[2t]; wc[2t+1]] (block 3: [wc[6]; 0])
    wc_d = nc.dram_tensor("wc", [2 * _CIN, 4 * _F], dt_st, kind="ExternalInput")
    b1_d = nc.dram_tensor("b1", [_F, 1], f32, kind="ExternalInput")
    # pre-transposed on host so each tile is one fully-contiguous HBM read.
    # Bulk: 7 groups of 4 pairs; tail: 2+1+1 pairs in shrinking transfers so
    # the final position-block's weights land (and the kernel can end) sooner.
    wla_d = nc.dram_tensor("wla", [7, _F, 4 * 2 * _K * _F], dt_st,
                           kind="ExternalInput")
    wlb_d = nc.dram_tensor("wlb", [_F, 2 * 2 * _K * _F], dt_st,
                           kind="ExternalInput")
    wlc_d = nc.dram_tensor("wlc", [_F, 2 * _K * _F], dt_st,
                           kind="ExternalInput")
    # final pair ships as two 7-chunk halves (no matmul straddles chunk 7)
    # so the very last dependency is a half-size transfer
    wld_d = nc.dram_tensor("wld", [2, _F, _K * _F], dt_st,
                           kind="ExternalInput")
    if bias_en:
        b2_d = nc.dram_tensor("b2", [1, _C * _F], dt_st, kind="ExternalInput")
    zdt = dt_st if mode == "bf16" else f32
    z_d = nc.dram_tensor("z", [_B, _C * _F], zdt, kind="ExternalOutput")

    Relu = mybir.ActivationFunctionType.Relu

    with tile.TileContext(nc) as tc:
        with (
            tc.tile_pool(name="const", bufs=2 if reps > 1 else 1) as cpool,
            tc.tile_pool(name="xt", bufs=2 if reps > 1 else 1) as xpool,
            tc.tile_pool(name="yt", bufs=1) as ypool,
            tc.tile_pool(name="wt", bufs=_WBUFS) as wpool,
            tc.tile_pool(name="wt2", bufs=2) as wpool2,
            # bias_en adds the 32KB/partition b2 row tile; drop zb double-
            # buffering to stay inside SBUF in that (untriggered here) case
            tc.tile_pool(name="zst", bufs=1 if bias_en else 2) as zpool,
            tc.tile_pool(name="psc", bufs=2, space="PSUM") as pscpool,
            tc.tile_pool(name="psl", bufs=4, space="PSUM") as pslpool,
        ):
            for r in range(reps):
                # ---- constants / inputs to SBUF (per rep: one rep == one
                # full kernel execution, every input re-read from HBM) ----
                # xt rides nc.sync ahead of the wl stream; tiny constant loads
                # go on nc.scalar so their issue latency overlaps the stream.
                wc_t = cpool.tile([2 * _CIN, 4 * _F], dt_st, tag="wc",
                                  name=f"wc{r}")
                nc.scalar.dma_start(wc_t[:], wc_d[:])
                b1_t = cpool.tile([_F, 1], f32, tag="b1", name=f"b1_{r}")
                nc.scalar.dma_start(b1_t[:], b1_d[:])
                if bias_en:
                    b2_t = cpool.tile([1, _C * _F], dt_st, tag="b2",
                                      name=f"b2_{r}")
                    nc.scalar.dma_start(b2_t[:], b2_d[:])
                    ones_t = cpool.tile([1, _B], dt_st, tag="ones",
                                        name=f"ones{r}")
                    nc.gpsimd.memset(ones_t[:], 1.0)

                xt_t = xpool.tile([2 * _CIN, (_LX + 1) * _B], dt_st, tag="xt",
                                  name=f"xt{r}")
                nxc = 2
                xch = ((_LX + 1) * _B) // nxc
                for c in range(nxc):
                    nc.sync.dma_start(
                        xt_t[:_CIN, c * xch:(c + 1) * xch],
                        xt_d[:, c * xch:(c + 1) * xch]
                    )
                # tap-shift duplicate: rows 64:127 col j = x[pos j+1]. An
                # SBUF->SBUF DMA (gpsimd queue: off the HBM weight stream)
                # moves across partitions; engines can't.
                nc.gpsimd.dma_start(
                    xt_t[_CIN:, :_LX * _B], xt_t[:_CIN, _B:])

                # ---- W stream (the big DMA) ----
                wgrps = []
                for gg in range(7):
                    wt = wpool.tile([_F, 4 * 2 * _K * _F], dt_st, tag="wt",
                                    name=f"wt{r}_{gg}")
                    nc.sync.dma_start(wt[:], wla_d[gg])
                    wgrps.append(wt)
                wtb = wpool2.tile([_F, 2 * 2 * _K * _F], dt_st, tag="wtb",
                                  name=f"wtb{r}")
                nc.sync.dma_start(wtb[:], wlb_d[:])
                wtc = wpool2.tile([_F, 2 * _K * _F], dt_st, tag="wtc",
                                  name=f"wtc{r}")
                nc.sync.dma_start(wtc[:], wlc_d[:])
                wtd = wpool2.tile([_F, 2 * _K * _F], dt_st, tag="wtd",
                                  name=f"wtd{r}")
                nc.sync.dma_start(wtd[:, :_K * _F], wld_d[0])
                nc.sync.dma_start(wtd[:, _K * _F:], wld_d[1])

                def wl_ap(g, c0, ncol):
                    if g < 28:
                        t_, base = wgrps[g // 4], (g % 4) * 2 * _K
                    elif g < 30:
                        t_, base = wtb, (g - 28) * 2 * _K
                    else:
                        t_, base = (wtc if g == 30 else wtd), 0
                    return t_[:, (base + c0) * _F:(base + c0 + ncol) * _F]

                # ---- conv + BN1 + ReLU -> yT [F, (j, b)] ----
                # taps paired two-per-matmul: stationary [wc[2t]; wc[2t+1]]
                # (128 rows), moving xt col j = [x[j]; x[j+1]] -> 4 mms per jb.
                yt_t = ypool.tile([_F, _NJB * 8 * _B], dt_st, tag="yt",
                                  name=f"yt{r}")
                for jb in range(_NJB):
                    ps = pscpool.tile([_F, 8 * _B], f32, tag="psc",
                                      name=f"psc{r}_{jb}")
                    for t in range(4):
                        nc.tensor.matmul(
                            ps[:],
                            cast(wc_t[:, t * _F:(t + 1) * _F]),
                            cast(xt_t[:, (8 * jb + 2 * t) * _B:
                                      (8 * jb + 2 * t + 8) * _B]),
                            start=(t == 0),
                            stop=(t == 3),
                        )
                    nc.scalar.activation(
                        yt_t[:, jb * 8 * _B:(jb + 1) * 8 * _B], ps[:], Relu,
                        bias=b1_t[:]
                    )

                # ---- locally-connected layer ----
                # bank-blocked: positions [4t, 4t+4) share one PSUM bank and
                # one accumulation group (HW start=True zeroes the whole 2KB
                # bank). wl cols: c = 2k + (p%2); at stationary q the active
                # chunks of a pair are adjacent -> one N=256 matmul. PSUM
                # sub-slot of local position j is j^1; host unpermutes.
                zb_t = zpool.tile([_B, _C * _F], zdt, tag="zb", name=f"zb{r}")
                for t in range(_C // 4):
                    ps = pslpool.tile([_B, 4 * _F], f32, tag="psl",
                                      name=f"psl{r}_{t}")
                    # singles first: the start=True MM marks the whole 2KB bank
                    # pending; the other three singles land in fully-pending
                    # slots; every later paired MM then touches only
                    # already-written bytes (uniform accumulate).
                    mms = [  # (q, g, col_lo, ncols, out_lo)
                        (4 * t, 2 * t, 0, 1, 1),
                        (4 * t + _K, 2 * t, 2 * _K - 1, 1, 0),
                        (4 * t + 2, 2 * t + 1, 0, 1, 3),
                        (4 * t + 2 + _K, 2 * t + 1, 2 * _K - 1, 1, 2),
                    ]
                    # paired MMs grouped by pair (g) so everything needing only
                    # the earlier-arriving pair runs before the later pair's
                    # DMA lands — shrinks the wait on the final position-block.
                    for g in (2 * t, 2 * t + 1):
                        for q in range(4 * t, 4 * t + 10):
                            ke, ko = q - 2 * g, q - 2 * g - 1
                            if 0 <= ko and ke < _K:      # both chunks active
                                mms.append((q, g, 2 * ke - 1, 2, 2 * g - 4 * t))
                    for i, (q, g, c0, ncol, u0) in enumerate(mms):
                        nc.tensor.matmul(
                            ps[:, u0 * _F:(u0 + ncol) * _F],
                            cast(yt_t[:, q * _B:(q + 1) * _B]),
                            cast(wl_ap(g, c0, ncol)),
                            start=(i == 0),
                            stop=(i == len(mms) - 1) and not bias_en,
                        )
                    base = 4 * t
                    if bias_en:
                        nc.tensor.matmul(
                            ps[:],
                            cast(ones_t[:, :_B]),
                            cast(b2_t[:, base * _F:(base + 4) * _F]),
                            start=False,
                            stop=True,
                            skip_group_check=True,
                        )
                    nc.scalar.activation(
                        zb_t[:, base * _F:(base + 4) * _F], ps[:], Relu)
                    # z leaves in three chunks (positions 0-31, 32-55, 56-63):
                    # the first two hide mid-kernel; only 0.13 MB rides the
                    # tail behind the last matmul's activation.
                    half, c2 = (_C // 2) * _F, (_C - 8) * _F
                    if t == _C // 8 - 1:
                        nc.scalar.dma_start(z_d[:, :half], zb_t[:, :half])
                    elif t == _C // 4 - 3:
                        nc.scalar.dma_start(z_d[:, half:c2], zb_t[:, half:c2])
                c2 = (_C - 8) * _F
                nc.scalar.dma_start(z_d[:, c2:], zb_t[:, c2:])
    nc.compile()
    return nc


def _host_prepare(x, conv_w, conv_b, bn1_gamma, bn1_beta, bn1_mean, bn1_var,
                  local_w, local_b, bn2_gamma, bn2_beta, bn2_mean, bn2_var,
                  mode: str | None = None):
    mode = mode or _MODE
    f = np.float32
    dt = _np_dt(mode)
    x = np.asarray(x, f)
    s1 = (np.asarray(bn1_gamma, f) / np.sqrt(np.asarray(bn1_var, f) + f(_EPS))).astype(f)
    wc = (np.asarray(conv_w, f) * s1[None, None, :]).astype(f)
    b1 = (s1 * (np.asarray(conv_b, f) - np.asarray(bn1_mean, f))
          + np.asarray(bn1_beta, f)).astype(f).reshape(_F, 1)
    s2 = (np.asarray(bn2_gamma, f) / np.sqrt(np.asarray(bn2_var, f) + f(_EPS))).astype(f)
    wl = (np.asarray(local_w, f) * s2[None, None, :]).astype(f)
    b2 = (s2[None, :] * (np.asarray(local_b, f) - np.asarray(bn2_mean, f)[None, :])
          + np.asarray(bn2_beta, f)[None, :]).astype(f)

    bias_en = bool(np.any(b2))

    npad = _NCORES * _C  # 512
    # pair-interleaved + F-transposed local_w: per pair [f, (c=2k+(p%2), n)]
    # so any run of pairs is one contiguous HBM read into its SBUF layout.
    wl_pad = np.zeros((npad, _K, _F, _F), f)
    wl_pad[:_OUT_LEN] = wl.reshape(_OUT_LEN, _K, _F, _F)
    wl_pT = np.ascontiguousarray(
        wl_pad.reshape(npad // 2, 2, _K, _F, _F)
        .transpose(0, 3, 2, 1, 4)             # [pair, f, k, p, n]
    ).reshape(npad // 2, _F, 2 * _K * _F).astype(dt)

    perm = np.arange(_C) ^ 1  # pair-swap (self-inverse)
    b2_pad = np.zeros((npad, _F), f)
    b2_pad[:_OUT_LEN] = b2

    # x padded for SAME conv + per-core halo: xpad[:, j] = x[:, j-3]
    xpad = np.zeros((_B, _L + 3 + 16, _CIN), f)
    xpad[:, 3:3 + _L] = x
    xpad = xpad.astype(dt)

    # tap-paired conv weights: [2*CIN, 4F]; block t = [wc[2t]; wc[2t+1]]
    wc128 = np.zeros((2 * _CIN, 4, _F), f)
    for t in range(3):
        wc128[:_CIN, t] = wc[2 * t]
        wc128[_CIN:, t] = wc[2 * t + 1]
    wc128[:_CIN, 3] = wc[6]
    wc128 = np.ascontiguousarray(wc128.reshape(2 * _CIN, 4 * _F)).astype(dt)

    in_maps = []
    for i in range(_NCORES):
        p0 = _C * i
        xs = xpad[:, p0:p0 + _LX + 1, :]                  # [B, LX+1, CIN]
        xt = np.ascontiguousarray(
            xs.transpose(2, 1, 0)                         # [CIN, LX+1, B]
        ).reshape(_CIN, (_LX + 1) * _B)
        pr = wl_pT[p0 // 2:p0 // 2 + _NPAIR]              # [32, F, 2K*F]
        wla = np.ascontiguousarray(
            pr[:28].reshape(7, 4, _F, 2 * _K * _F).transpose(0, 2, 1, 3)
        ).reshape(7, _F, 4 * 2 * _K * _F)
        wlb = np.ascontiguousarray(
            pr[28:30].transpose(1, 0, 2)).reshape(_F, 2 * 2 * _K * _F)
        wld = np.ascontiguousarray(
            pr[31].reshape(_F, 2, _K * _F).transpose(1, 0, 2))
        m = {"xt": xt, "wc": wc128, "b1": b1,
             "wla": wla, "wlb": wlb, "wlc": pr[30], "wld": wld}
        if bias_en:
            m["b2"] = np.ascontiguousarray(
                b2_pad[p0:p0 + _C][perm].reshape(1, _C * _F)).astype(dt)
        in_maps.append(m)
    return in_maps, bias_en


def _assemble(results):
    f = np.float32
    perm = np.arange(_C) ^ 1
    z = np.empty((_B, _OUT_LEN, _F), f)
    for i in range(_NCORES):
        p0 = _C * i
        zi = np.asarray(results[i]["z"], f).reshape(_B, _C, _F)[:, perm]
        n = min(_C, _OUT_LEN - p0)
        z[:, p0:p0 + n] = zi[:, :n]
    return z


def kernel(**inputs) -> np.ndarray:
    from concourse.bass_utils import run_bass_kernel_spmd

    in_maps, bias_en = _host_prepare(**inputs)
    nc = _build_program(bias_en)
    res = run_bass_kernel_spmd(nc, in_maps, list(range(_NCORES)))
    return _assemble(res.results)

